# revision 2
# baseline (speedup 1.0000x reference)
"""MHA on 8 TRN2 cores — pipelined per-chunk ReduceScatter variant.

Sharding: core c -> (batch b = c//4, head-group g = c%4). Each core applies
its OWN 256 rows of the contraction in the output projection directly from
SBUF, and the 4 cores of a batch group ReduceScatter each 512-row q-chunk's
bf16 partial y [512, D] as soon as its out-projection lands -> 4 small
collectives per iteration instead of one big one, each overlapping the next
chunk's attention compute. Collective outputs live in the Shared scratchpad
(fast HBM-HBM path). Core g keeps rows [512c+128g : 512c+128g+128) of the
summed y for every chunk c.
"""

import numpy as np
import ml_dtypes

import concourse.bass as bass
import concourse.mybir as mybir
import concourse.tile as tile
from concourse.bass_utils import run_bass_kernel_spmd

BF16 = ml_dtypes.bfloat16
F32 = mybir.dt.float32
BF = mybir.dt.bfloat16

B, S, D, H = 2, 2048, 1024, 16
DK = D // H
HPC = H // 4
EG = D // 4
KT = D // 128
GROUPS = [[0, 1, 2, 3], [4, 5, 6, 7]]
EXP = mybir.ActivationFunctionType.Exp

TRACE = False
LAST_EXEC_NS = None


# --- workaround: this walrus build only encodes ONE sync wait per
# instruction ("Too many sync wait commands" in setupSyncWait). Hoist
# excess waits onto same-engine NOP carriers placed just before the
# instruction; engines execute in order, so semantics are unchanged. ---
def _split_multi_waits(nc, max_waits=1):
    n = 0
    for f in nc.m.functions:
        for bb in f.blocks:
            new = []
            for inst in bb.instructions:
                si = inst.sync_info
                waits = list(si.on_wait) if si is not None and si.on_wait else []
                if len(waits) > max_waits:
                    keep = len(waits) - max_waits
                    for j in range(0, keep, max_waits):
                        n += 1
                        new.append(
                            mybir.InstNoOp(
                                name=f"waitsplit-{n}",
                                engine=inst.engine,
                                bass_nofuse=True,
                                sync_info=mybir.SyncInfo(
                                    on_wait=waits[j : j + max_waits], on_update=[]
                                ),
                            )
                        )
                    si.on_wait = waits[keep:]
                new.append(inst)
            bb.instructions[:] = new
    return n


def _bf16_c(a):
    return np.ascontiguousarray(a).astype(BF16)



def build(s=S, repeat=1):
    n_sc = s // 512
    n_st = s // 128

    nc = bass.Bass(num_devices=8)
    xq_t = nc.declare_dram_parameter("xq_t", [D, s], BF, isOutput=False)
    xk_t = nc.declare_dram_parameter("xk_t", [D, s], BF, isOutput=False)
    xv_t = nc.declare_dram_parameter("xv_t", [D, s], BF, isOutput=False)
    wq_t = nc.declare_dram_parameter("wq_t", [D, EG], BF, isOutput=False)
    wk_t = nc.declare_dram_parameter("wk_t", [D, EG], BF, isOutput=False)
    wv_t = nc.declare_dram_parameter("wv_t", [D, EG], BF, isOutput=False)
    # Wo slice for THIS core's attn dims: [EG rows = local dims, D out cols]
    wo_t = nc.declare_dram_parameter("wo_t", [EG, D], BF, isOutput=False)
    # output: this core's q-row slice of the reduced y, in bf16
    y_ext = nc.declare_dram_parameter("y", [s // 4, D], BF, isOutput=True)

    bounce = [nc.dram_tensor(f"py_bounce{par}", [s, D], BF) for par in range(2)]
    rs_out = [nc.dram_tensor(f"py_rs{par}", [s // 4, D], BF) for par in range(2)]

    with tile.TileContext(nc) as tc:
        with (
            tc.tile_pool(name="kvp", bufs=2) as kvp,
            tc.tile_pool(name="wpool", bufs=1) as wp,
            tc.tile_pool(name="xpool", bufs=2) as xp,
            tc.tile_pool(name="psum2", bufs=1, space="PSUM") as ps2,
            tc.tile_pool(name="expp", bufs=3) as ep,
            tc.tile_pool(name="normp", bufs=2) as np_,
            tc.tile_pool(name="qcp", bufs=2) as qcp,
            tc.tile_pool(name="acp", bufs=2) as acp,
            tc.tile_pool(name="yp", bufs=2) as yp,
        ):
            wts = {}
            for nm, src in (("wq", wq_t), ("wk", wk_t), ("wv", wv_t)):
                wts[nm] = wp.tile([128, KT, EG], BF, tag=nm, name=nm)
                nc.sync.dma_start(
                    wts[nm][:], src[:].rearrange("(k p) c -> p k c", k=KT)
                )
            wq, wk, wv = wts["wq"], wts["wk"], wts["wv"]
            # wo: [128 local-dims, 2 pair-blocks, D]: block p = dims of heads
            # (2p, 2p+1)
            wo_sb = wp.tile([128, 2, D], BF, tag="wo", name="wo")
            nc.sync.dma_start(
                wo_sb[:], wo_t[:].rearrange("(p r) c -> r p c", p=2)
            )

            def emit_op(c, attnc, par):
                """Deferred partial out-projection for chunk c from its
                (still-live) attnc tiles, + bounce write + this chunk's
                ReduceScatter and output-slice DMA (pipelined collective)."""
                cs = slice(c * 512, (c + 1) * 512)
                rs = slice(c * 128, (c + 1) * 128)
                ysb = yp.tile([128, 4, 2, 512], BF, tag="ysb", name=f"ysb{c}")
                for qt in range(4):
                    for half in range(2):
                        yps = ps2.tile([128, 512], F32, tag="p1", bufs=2,
                                       name=f"yps{c}_{qt}_{half}")
                        for i, pair in enumerate((0, 1)):
                            nc.tensor.matmul(
                                yps[:],
                                attnc[pair][:, qt * 128:(qt + 1) * 128],
                                wo_sb[:, pair, half * 512:(half + 1) * 512],
                                start=(i == 0),
                                stop=(i == 1),
                            )
                        nc.vector.tensor_copy(ysb[:, qt, half, :], yps[:])
                nc.sync.dma_start(
                    bounce[par][cs, :]
                    .rearrange("(qt p) (h c) -> p qt h c", qt=4, h=2),
                    ysb[:],
                )
                nc.gpsimd.collective_compute(
                    "ReduceScatter",
                    mybir.AluOpType.add,
                    replica_groups=GROUPS,
                    ins=[bounce[par][cs, :]],
                    outs=[rs_out[par][rs, :]],
                )
                nc.sync.dma_start(y_ext[rs, :], rs_out[par][rs, :])

            pend = None  # (chunk, attnc tiles, parity) awaiting out-proj
            for _rep in range(repeat):
                par = _rep % 2
                kTc = [[None] * n_sc, [None] * n_sc]
                vE = [None] * n_st
                for c2 in range(n_sc):
                    cs2 = slice(c2 * 512, (c2 + 1) * 512)
                    xk = xp.tile([128, KT, 512], BF, tag="xk", name=f"xk{_rep}_{c2}")
                    xv = xp.tile([128, KT, 512], BF, tag="xv", name=f"xv{_rep}_{c2}")
                    nc.sync.dma_start(
                        xk[:], xk_t[:, cs2].rearrange("(k p) c -> p k c", k=KT)
                    )
                    nc.sync.dma_start(
                        xv[:], xv_t[:, cs2].rearrange("(k p) c -> p k c", k=KT)
                    )
                    for e in range(2):
                        ps = ps2.tile([128, 512], F32, tag="p1", bufs=2,
                                      name=f"pk{_rep}_{c2}{e}")
                        for k in range(KT):
                            nc.tensor.matmul(
                                ps[:],
                                wk[:, k, e * 128:(e + 1) * 128],
                                xk[:, k, :],
                                start=(k == 0),
                                stop=(k == KT - 1),
                            )
                        kTc[e][c2] = kvp.tile([128, 512], BF, tag=f"kTc{e}_{c2}",
                                              name=f"kTc{_rep}_{e}_{c2}")
                        nc.vector.tensor_copy(kTc[e][c2][:], ps[:])
                    for t in range(4 * c2, 4 * c2 + 4):
                        tl = slice((t % 4) * 128, (t % 4) * 128 + 128)
                        ps = ps2.tile([128, EG], F32, tag="p1", bufs=2,
                                      name=f"pv{_rep}_{t}")
                        for k in range(KT):
                            nc.tensor.matmul(
                                ps[:],
                                xv[:, k, tl],
                                wv[:, k, :],
                                start=(k == 0),
                                stop=(k == KT - 1),
                            )
                        vE[t] = kvp.tile([128, HPC * 2 * DK], BF, tag=f"vE{t}",
                                         name=f"vE{_rep}_{t}")
                        nc.vector.memset(vE[t][:], 1.0)
                        for h in range(HPC):
                            nc.vector.tensor_copy(
                                vE[t][:, h * 2 * DK:h * 2 * DK + DK],
                                ps[:, h * DK:(h + 1) * DK],
                            )
                    if c2 == 1 and pend is not None:
                        # previous repeat's chunk-3 out-projection + its RS
                        pc, pattnc, ppar = pend
                        emit_op(pc, pattnc, ppar)
                        pend = None

                for c in range(n_sc):
                    cs = slice(c * 512, (c + 1) * 512)
                    xq = xp.tile([128, KT, 512], BF, tag="xq", name=f"xq{_rep}_{c}")
                    nc.sync.dma_start(
                        xq[:], xq_t[:, cs].rearrange("(k p) c -> p k c", k=KT)
                    )
                    qp = ps2.tile([128, 1024], F32, tag="scores", bufs=2,
                                  name=f"qp{_rep}_{c}")
                    for e in range(2):
                        for k in range(KT):
                            nc.tensor.matmul(
                                qp[:, e * 512:(e + 1) * 512],
                                wq[:, k, e * 128:(e + 1) * 128],
                                xq[:, k, :],
                                start=(k == 0),
                                stop=(k == KT - 1),
                            )
                    qTc = [qcp.tile([128, 512], BF, tag=f"qTc{e}",
                                    name=f"qTc{_rep}_{c}_{e}") for e in range(2)]
                    for e in range(2):
                        nc.vector.tensor_copy(qTc[e][:], qp[:, e * 512:(e + 1) * 512])

                    attnc = [acp.tile([128, 512], BF, tag=f"attnc{t2}",
                                      name=f"attnc{_rep}_{c}_{t2}") for t2 in range(2)]
                    for pair in range(2):
                        aP = [ps2.tile([128, 512], F32, tag=f"attnP{sub}",
                                       name=f"aP{_rep}_{c}_{pair}_{sub}")
                              for sub in range(2)]
                        for t in range(n_st):
                            scp = ps2.tile([128, 1024], F32, tag="scores", bufs=2,
                                           name=f"sc{_rep}_{c}_{pair}_{t}")
                            for sub in range(2):
                                row = slice(64 * sub, 64 * sub + 64)
                                nc.tensor.matmul(
                                    scp[:, sub * 512:(sub + 1) * 512],
                                    kTc[pair][t // 4][row, (t % 4) * 128:(t % 4) * 128 + 128],
                                    qTc[pair][row, :],
                                    start=True,
                                    stop=True,
                                )
                            ex = ep.tile([128, 1024], BF, tag="expT", bufs=3,
                                         name=f"ex{_rep}_{c}_{pair}_{t}")
                            nc.scalar.activation(ex[:], scp[:], EXP, scale=1.0 / 8.0)
                            for sub in range(2):
                                h = 2 * pair + sub
                                nc.tensor.matmul(
                                    aP[sub][:],
                                    vE[t][:, h * 2 * DK:(h + 1) * 2 * DK],
                                    ex[:, sub * 512:(sub + 1) * 512],
                                    start=(t == 0),
                                    stop=(t == n_st - 1),
                                )
                        for sub in range(2):
                            den = np_.tile([DK, 512], F32, tag="den",
                                           name=f"den{_rep}_{c}_{pair}_{sub}")
                            nc.vector.reciprocal(den[:], aP[sub][DK:2 * DK, :])
                            nc.vector.tensor_mul(
                                attnc[pair][64 * sub:64 * sub + 64, :],
                                aP[sub][0:DK, :],
                                den[:],
                            )
                    # out-projection deferred one chunk: emit chunk c-1's now
                    # (its attnc tiles are the other acp buffers, still live)
                    if c >= 1:
                        emit_op(c - 1, prev_attnc, par)
                    prev_attnc = attnc
                pend = (n_sc - 1, prev_attnc, par)
            if pend is not None:
                pc, pattnc, ppar = pend
                emit_op(pc, pattnc, ppar)

    _split_multi_waits(nc)
    return nc


def make_maps(inputs):
    maps = []
    for c in range(8):
        b, g = divmod(c, 4)
        eg = slice(EG * g, EG * (g + 1))
        maps.append(
            {
                "xq_t": _bf16_c(np.asarray(inputs["query"], np.float32)[b].T),
                "xk_t": _bf16_c(np.asarray(inputs["key"], np.float32)[b].T),
                "xv_t": _bf16_c(np.asarray(inputs["value"], np.float32)[b].T),
                "wq_t": _bf16_c(np.asarray(inputs["Wq"], np.float32)[eg].T),
                "wk_t": _bf16_c(np.asarray(inputs["Wk"], np.float32)[eg].T),
                "wv_t": _bf16_c(np.asarray(inputs["Wv"], np.float32)[eg].T),
                "wo_t": _bf16_c(np.asarray(inputs["Wo"], np.float32)[:, eg].T),
            }
        )
    return maps


def assemble(results):
    """results: list of 8 per-core {'y': [S//4, D] bf16} -> full [B,S,D] f32.

    Core (b, g) row 128*ch + j holds y[b, 512*ch + 128*g + j] (per-chunk RS:
    chunk ch's 512 rows are scattered 128-apiece across the 4 group cores).
    """
    y = np.empty((B, S, D), np.float32)
    for c in range(8):
        b, g = divmod(c, 4)
        yc = np.asarray(results[c]["y"], np.float32).reshape(4, 128, D)
        for ch in range(4):
            y[b][512 * ch + 128 * g:512 * ch + 128 * g + 128, :] = yc[ch]
    return y


def kernel(query, key, value, Wq, bq, Wk, bk, Wv, bv, Wo, bo):
    global LAST_EXEC_NS
    for b_ in (bq, bk, bv, bo):
        assert not np.any(np.asarray(b_)), "nonzero biases not supported"
    nc = build(S)
    in_maps = make_maps(
        dict(query=query, key=key, value=value, Wq=Wq, Wk=Wk, Wv=Wv, Wo=Wo)
    )
    # retries: the axon tunnel occasionally drops a run with a transient
    # "mesh desynced" / NRT error, and the degraded device has been seen to
    # return NaN-corrupted results once in ~15 otherwise-identical runs; a
    # fresh dispatch succeeds. Inputs here are finite, so NaN in the output
    # always indicates a corrupted run, never valid data.
    import time as _time

    y = None
    for attempt in range(3):
        try:
            res = run_bass_kernel_spmd(nc, in_maps, list(range(8)), trace=TRACE)
        except Exception:
            if attempt == 2:
                raise
            _time.sleep(10)
            continue
        LAST_EXEC_NS = res.exec_time_ns
        y = assemble(res.results)
        if np.isfinite(y).all():
            break
        _time.sleep(5)
    return y

